# revision 30
# baseline (speedup 1.0000x reference)
"""Trainium2 Bass kernel for EnhancedTripletLoss (hard-mining triplet loss).

Flip-orientation design (8 NeuronCores, SPMD, no collectives):
  * Core c owns anchors of class c (8 classes == 8 cores), CAPPED at 1024
    rows (8 slabs of 128).  The handful of anchors beyond 1024 per class
    (class counts run ~1008..1040) are folded in exactly on the host in
    float64 -- removing the 9th, mostly-padding slab keeps every PSUM tile
    at exactly 2 banks so the pool can triple-buffer.
  * The distance work is computed TRANSPOSED vs the obvious layout:
    candidates sit on SBUF partitions (stationary matmul operand, 128 per
    tile) and the core's 1024 anchors stream along the matmul free dim.
  * psum[cand, a] = sum_k (2*a_k)*(e_cand_k) via 2 bf16 matmuls (K=256 in
    two 128-row halves).  Candidate tiles: own class first (pos tiles,
    padded with duplicate columns), then all other classes packed densely
    (neg tiles).  Mining needs min over tiles of (psum -+ sq'_cand) where
    sq' = ||e||^2 - 256 rides as a per-PARTITION scalar (exact fp32) -- no
    third matmul channel, unlike the row-oriented layout.
  * Each tile's PSUM is drained by BOTH engines concurrently to keep up
    with the PE: the Scalar engine copies anchors [0:FA] through
    Identity(x + bias) into fp16, the DVE folds the tail [FA:1024] straight
    from PSUM via scalar_tensor_tensor(add, min).  The fp16 fold of the ACT
    piece (tensor_tensor min, 2x 16-bit throughput) runs one tile behind so
    it never waits.  fp16 mining noise perturbs the loss ~5e-7 relative
    (simulated): extreme-value errors average out over 8192 anchors.
  * Endgame: accumulators live transposed ([cand-lane, anchor]); PE
    identity-matmul transposes flip each 128-anchor slab and one segmented
    DVE reduce extracts per-anchor minima.  The positive endgame is emitted
    mid-stream; only the negative endgame sits in the tail.
  * Each core writes per-partition partials [128, 2] (loss numerator,
    valid count); the host adds the excess-anchor terms and divides.
"""

import numpy as np
import ml_dtypes

P = 128
D = 256
NCLS = 8
NCORES = 8
MARGIN = 0.3
F16INIT = 60000.0    # fp16 +inf stand-in for min-accumulators
CENTER = 256.0       # ||e||^2 recentering constant (D=256)
SLABS = 8            # anchor slabs per core (1024 anchors; excess -> host)
MC = SLABS * P
FA = 928             # anchor cols consumed by the ACT path (rest: DVE STT)

BF16 = ml_dtypes.bfloat16


def _layout(counts):
    """(tpos, tneg) candidate-tile counts from class counts.

    tpos is capped at SLABS tiles (1024 candidates); the few positive
    candidates beyond that are max-merged on the host from the raw
    per-anchor stats the kernel ships out.
    """
    B = int(np.sum(counts))
    tpos = min(SLABS, max(max(1, -(-int(n) // P)) for n in counts))
    tneg = max(-(-(B - int(n)) // P) for n in counts)
    return tpos, tneg


def _chunks(T):
    """DMA chunk sizes (in cand tiles): small first for a fast start."""
    out = [3, 3, 3]
    rest = T - 9
    while rest > 0:
        w = min(6, rest)
        out.append(w)
        rest -= w
    return out


def _build_program(tpos, tneg):
    import concourse.tile as tile
    from concourse import bacc, mybir

    f32 = mybir.dt.float32
    f16 = mybir.dt.float16
    bf16 = mybir.dt.bfloat16
    AX = mybir.AxisListType.X
    OP = mybir.AluOpType
    IDENT = mybir.ActivationFunctionType.Identity

    T = tpos + tneg
    NCOLS = T * P
    chunks = _chunks(T)

    nc = bacc.Bacc("TRN2", target_bir_lowering=False, debug=False)

    vmat = nc.dram_tensor("vmat", [P, 2 * NCOLS], bf16, kind="ExternalInput")
    umat = nc.dram_tensor("umat", [P, 2 * MC], bf16, kind="ExternalInput")
    ssqt = nc.dram_tensor("ssqt", [P, T], f32, kind="ExternalInput")
    out = nc.dram_tensor("out", [P, 2 * MC], f16, kind="ExternalOutput")

    with tile.TileContext(nc) as tc:
        with (
            tc.tile_pool(name="resident", bufs=1) as res,
            tc.tile_pool(name="psum", bufs=3, space="PSUM") as pp,
            tc.tile_pool(name="fold", bufs=4) as fp,
        ):
            # ---- PE warmup while DMA fills --------------------------------
            wsrc = res.tile([P, 512], bf16, tag="wsrc")
            nc.vector.memset(wsrc[:], 0.0)
            wp = pp.tile([P, MC], f32, tag="pblk", name="warm")
            for _ in range(8):
                nc.tensor.matmul(wp[:, 0:512], wsrc[:, 0:P], wsrc[:, :],
                                 start=True, stop=True)

            # ---- DMA fill (sync + gpsimd queues; scalar stays free) -------
            dma_engs = [nc.sync, nc.gpsimd]
            _rr = [0]

            def dma(out_ap, in_ap):
                dma_engs[_rr[0] % 2].dma_start(out=out_ap, in_=in_ap)
                _rr[0] += 1

            ut = res.tile([P, 2, MC], bf16, tag="umat")
            for h in (0, 1):
                dma(ut[:, h, :], umat[:, h * MC:(h + 1) * MC])
            ssqt_t = res.tile([P, T], f32, tag="ssqt")
            nc.scalar.dma_start(out=ssqt_t[:], in_=ssqt[:, :])

            vts = []       # per cand-tile: (chunk_tile, col_offset_in_chunk)
            coff = 0
            for ci, cw in enumerate(chunks):
                vt = res.tile([P, 2, cw * P], bf16, tag=f"v{ci}",
                              name=f"v{ci}")
                for h in (0, 1):
                    o = h * NCOLS + coff * P
                    dma(vt[:, h, :], vmat[:, o:o + cw * P])
                for j in range(cw):
                    vts.append((vt, j))
                coff += cw

            accP = res.tile([P, MC], f16, tag="accP")
            nc.vector.memset(accP[:], F16INIT)
            accN = res.tile([P, MC], f16, tag="accN")
            nc.vector.memset(accN[:], F16INIT)

            segs = [(0, 512), (512, 512)]

            # ---- main loop over candidate tiles ---------------------------
            pend = None  # (tmp, acc) waiting for its fp16 fold
            for t in range(T):
                vt, j = vts[t]
                psum = pp.tile([P, MC], f32, tag="pblk", name="pblk")
                for h in (0, 1):
                    stat = vt[:, h, j * P:(j + 1) * P]
                    for (s0, sw) in segs:
                        nc.tensor.matmul(
                            psum[:, s0:s0 + sw], stat, ut[:, h, s0:s0 + sw],
                            start=(h == 0), stop=(h == 1),
                        )
                acc = accP if t < tpos else accN
                tmp = fp.tile([P, FA], f16, tag="fold")
                nc.scalar.activation(tmp[:], psum[:, 0:FA], IDENT,
                                     bias=ssqt_t[:, t:t + 1], scale=1.0)
                nc.vector.scalar_tensor_tensor(
                    acc[:, FA:MC], in0=psum[:, FA:MC],
                    scalar=ssqt_t[:, t:t + 1],
                    in1=acc[:, FA:MC], op0=OP.add, op1=OP.min,
                )
                if pend is not None:
                    ptmp, pacc = pend
                    nc.vector.tensor_tensor(pacc[:, 0:FA], ptmp[:],
                                            pacc[:, 0:FA], op=OP.min)
                pend = (tmp, acc)
                if t == tpos + 1:
                    nc.sync.dma_start(out=out[:, 0:MC], in_=accP[:])

            ptmp, pacc = pend
            nc.vector.tensor_tensor(pacc[:, 0:FA], ptmp[:], pacc[:, 0:FA],
                                    op=OP.min)
            nc.sync.dma_start(out=out[:, MC:2 * MC], in_=accN[:])

    nc.compile()
    return nc


def _prepare_inputs(emb, lab, tpos, tneg):
    """Host-side shard/layout prep.

    Returns (in_maps, excess) where excess is the list of anchor indices
    (per class beyond the 1024-row cap) to be folded in on the host.
    """
    B = emb.shape[0]
    T = tpos + tneg
    NCOLS = T * P

    counts = np.bincount(lab, minlength=NCLS).astype(int)
    order = np.argsort(lab, kind="stable")
    cstart = np.concatenate([[0], np.cumsum(counts)]).astype(int)
    sq = np.einsum("ij,ij->i", emb, emb, dtype=np.float32)
    sqp = sq - np.float32(CENTER)

    in_maps = []
    excess = []
    meta = []
    for c in range(NCLS):
        own = order[cstart[c]:cstart[c + 1]]
        if len(own) == 0:
            own = order[0:1]
        aidx = own[:MC]
        excess.extend(own[MC:].tolist())
        aidx_p = np.concatenate(
            [aidx, np.full(MC - len(aidx), aidx[0], dtype=np.int64)])

        # candidate columns: own class (padded/capped) first, then others
        own_cap = own[:tpos * P]
        pos_cols = np.concatenate(
            [own_cap,
             np.full(tpos * P - len(own_cap), own[0], dtype=np.int64)])
        negs = np.concatenate(
            [order[cstart[c2]:cstart[c2 + 1]]
             for c2 in range(NCLS) if c2 != c and counts[c2] > 0])
        neg_cols = np.concatenate(
            [negs, np.full(tneg * P - len(negs), negs[0], dtype=np.int64)])
        colidx = np.concatenate([pos_cols, neg_cols])
        assert len(colidx) == NCOLS

        sign = np.ones(NCOLS, dtype=np.float32)
        sign[tpos * P:] = -1.0

        V = (emb[colidx] * sign[:, None]).astype(BF16).T  # [256, NCOLS]
        vm = np.empty((P, 2 * NCOLS), dtype=BF16)
        vm[:, :NCOLS] = V[0:P]
        vm[:, NCOLS:] = V[P:2 * P]

        U = (2.0 * emb[aidx_p]).astype(BF16).T            # [256, MC]
        um = np.empty((P, 2 * MC), dtype=BF16)
        um[:, :MC] = U[0:P]
        um[:, MC:] = U[P:2 * P]

        # per-tile scalar column: -sq' for pos tiles, +sq' for neg tiles
        ssqcol = np.where(sign > 0, -sqp[colidx], sqp[colidx])
        ssqt = ssqcol.reshape(T, P).T.copy()              # [128, T]

        in_maps.append({"vmat": vm, "umat": um, "ssqt": ssqt})
        meta.append((c, aidx, own[tpos * P:]))
    return in_maps, excess, meta


def _host_excess(emb, lab, excess):
    """Exact float64 triplet terms for anchors beyond the per-core cap."""
    if not excess:
        return 0.0, 0
    e64 = emb.astype(np.float64)
    sq64 = np.einsum("ij,ij->i", e64, e64)
    num = 0.0
    n = 0
    B = emb.shape[0]
    counts = np.bincount(lab, minlength=NCLS)
    for a in excess:
        c = lab[a]
        if not (2 <= counts[c] <= B - 1):
            continue
        d2 = sq64[a] + sq64 - 2.0 * (e64 @ e64[a])
        same = lab == c
        pos_d2 = np.max(np.where(same, d2, -np.inf))   # self d2=0 never max
        neg_d2 = np.min(np.where(same, np.inf, d2))
        per = np.sqrt(max(pos_d2, 0.0)) - np.sqrt(max(neg_d2, 0.0)) + MARGIN
        num += max(per, 0.0)
        n += 1
    return num, n


_PROGRAM_CACHE = {}


def _get_program(tpos, tneg):
    key = (tpos, tneg, FA)
    if key not in _PROGRAM_CACHE:
        _PROGRAM_CACHE[key] = _build_program(tpos, tneg)
    return _PROGRAM_CACHE[key]


def _combine(results, meta, emb, lab, counts, extra_num, extra_den):
    """Host epilogue: stats -> distances -> masked triplet loss."""
    B = emb.shape[0]
    sq = np.einsum("ij,ij->i", emb.astype(np.float64), emb.astype(np.float64))
    num = float(extra_num)
    den = float(extra_den)
    for r, (c, aidx, exc) in zip(results, meta):
        if not (2 <= counts[c] <= B - 1) or len(aidx) == 0:
            continue
        o = np.asarray(r["out"]).astype(np.float64)      # [128, 2*MC] fp16
        n = len(aidx)
        posst = o[:, 0:MC].min(axis=0)[:n]               # min over cand lanes
        negst = o[:, MC:2 * MC].min(axis=0)[:n]
        sqa = sq[aidx]
        pos_d2 = sqa + CENTER - posst
        neg_d2 = sqa + CENTER + negst
        if len(exc):                                     # uncovered positives
            d2x = (sqa[:, None] + sq[exc][None, :]
                   - 2.0 * emb[aidx].astype(np.float64)
                   @ emb[exc].astype(np.float64).T)
            pos_d2 = np.maximum(pos_d2, d2x.max(axis=1))
        per = (np.sqrt(np.maximum(pos_d2, 0.0))
               - np.sqrt(np.maximum(neg_d2, 0.0)) + MARGIN)
        num += np.maximum(per, 0.0).sum()
        den += n
    return np.float32(num / max(den, 1.0))


def _setup_trace_hook():
    import sys
    import types
    try:
        from antenv.axon_hooks import get_axon_ntff_profile_hook  # noqa: F401
        return
    except ImportError:
        pass
    import antenv
    from trn_agent_boot.trn_boot import _ntff_profile_via_ctypes

    mod = types.ModuleType("antenv.axon_hooks")
    state = {"h": None}
    mod.set_axon_ntff_profile_hook = lambda h: state.__setitem__("h", h)
    mod.get_axon_ntff_profile_hook = lambda: state["h"]
    sys.modules["antenv.axon_hooks"] = mod
    antenv.axon_hooks = mod
    mod.set_axon_ntff_profile_hook(
        _ntff_profile_via_ctypes("/opt/axon/libaxon_pjrt.so")
    )


def kernel(embeddings, labels, _trace=False):
    emb = np.ascontiguousarray(np.asarray(embeddings, dtype=np.float32))
    lab = np.asarray(labels).astype(np.int64).ravel()

    counts = np.bincount(lab, minlength=NCLS).astype(int)
    tpos, tneg = _layout(counts)
    nc = _get_program(tpos, tneg)
    in_maps, excess, meta = _prepare_inputs(emb, lab, tpos, tneg)
    extra_num, extra_den = _host_excess(emb, lab, excess)

    from concourse.bass_utils import run_bass_kernel_spmd

    if _trace:
        _setup_trace_hook()
        import concourse.bass_utils as _bu
        _bu.upload_artifacts = lambda tmpdir: tmpdir  # skip remote upload

    res = run_bass_kernel_spmd(
        nc, in_maps, core_ids=list(range(NCORES)), trace=bool(_trace),
    )
    loss = _combine(res.results, meta, emb, lab, counts,
                    extra_num, extra_den)
    if _trace:
        return loss, res
    return loss


# revision 31
# speedup vs baseline: 1.0427x; 1.0427x over previous
"""Trainium2 Bass kernel for EnhancedTripletLoss (hard-mining triplet loss).

Flip-orientation design (8 NeuronCores, SPMD, no collectives):
  * Core c owns anchors of class c (8 classes == 8 cores), CAPPED at 1024
    rows (8 slabs of 128).  The handful of anchors beyond 1024 per class
    (class counts run ~1008..1040) are folded in exactly on the host in
    float64 -- removing the 9th, mostly-padding slab keeps every PSUM tile
    at exactly 2 banks so the pool can triple-buffer.
  * The distance work is computed TRANSPOSED vs the obvious layout:
    candidates sit on SBUF partitions (stationary matmul operand, 128 per
    tile) and the core's 1024 anchors stream along the matmul free dim.
  * psum[cand, a] = sum_k (2*a_k)*(e_cand_k) via 2 bf16 matmuls (K=256 in
    two 128-row halves).  Candidate tiles: own class first (pos tiles,
    padded with duplicate columns), then all other classes packed densely
    (neg tiles).  Mining needs min over tiles of (psum -+ sq'_cand) where
    sq' = ||e||^2 - 256 rides as a per-PARTITION scalar (exact fp32) -- no
    third matmul channel, unlike the row-oriented layout.
  * Each tile's PSUM is drained by BOTH engines concurrently to keep up
    with the PE: the Scalar engine copies anchors [0:FA] through
    Identity(x + bias) into fp16, the DVE folds the tail [FA:1024] straight
    from PSUM via scalar_tensor_tensor(add, min).  The fp16 fold of the ACT
    piece (tensor_tensor min, 2x 16-bit throughput) runs one tile behind so
    it never waits.  fp16 mining noise perturbs the loss ~5e-7 relative
    (simulated): extreme-value errors average out over 8192 anchors.
  * Endgame: accumulators live transposed ([cand-lane, anchor]); PE
    identity-matmul transposes flip each 128-anchor slab and one segmented
    DVE reduce extracts per-anchor minima.  The positive endgame is emitted
    mid-stream; only the negative endgame sits in the tail.
  * Each core writes per-partition partials [128, 2] (loss numerator,
    valid count); the host adds the excess-anchor terms and divides.
"""

import numpy as np
import ml_dtypes

P = 128
D = 256
NCLS = 8
NCORES = 8
MARGIN = 0.3
F16INIT = 60000.0    # fp16 +inf stand-in for min-accumulators
CENTER = 256.0       # ||e||^2 recentering constant (D=256)
SLABS = 8            # anchor slabs per core (1024 anchors; excess -> host)
MC = SLABS * P
FA = 864             # anchor cols consumed by the ACT path (rest: DVE STT)

BF16 = ml_dtypes.bfloat16


def _layout(counts):
    """(tpos, tneg) candidate-tile counts from class counts.

    tpos is capped at SLABS tiles (1024 candidates); the few positive
    candidates beyond that are max-merged on the host from the raw
    per-anchor stats the kernel ships out.
    """
    B = int(np.sum(counts))
    tpos = min(SLABS, max(max(1, -(-int(n) // P)) for n in counts))
    tneg = max(-(-(B - int(n)) // P) for n in counts)
    return tpos, tneg


def _chunks(T):
    """DMA chunk sizes (in cand tiles): small first for a fast start."""
    out = [3, 3, 3]
    rest = T - 9
    while rest > 0:
        w = min(6, rest)
        out.append(w)
        rest -= w
    return out


def _build_program(tpos, tneg):
    import concourse.tile as tile
    from concourse import bacc, mybir

    f32 = mybir.dt.float32
    f16 = mybir.dt.float16
    bf16 = mybir.dt.bfloat16
    AX = mybir.AxisListType.X
    OP = mybir.AluOpType
    IDENT = mybir.ActivationFunctionType.Identity

    T = tpos + tneg
    NCOLS = T * P
    chunks = _chunks(T)

    nc = bacc.Bacc("TRN2", target_bir_lowering=False, debug=False)

    vmat = nc.dram_tensor("vmat", [P, 2 * NCOLS], bf16, kind="ExternalInput")
    umat = nc.dram_tensor("umat", [P, 2 * MC], bf16, kind="ExternalInput")
    ssqt = nc.dram_tensor("ssqt", [P, T], f32, kind="ExternalInput")
    out = nc.dram_tensor("out", [P, 2 * MC], f16, kind="ExternalOutput")

    with tile.TileContext(nc) as tc:
        with (
            tc.tile_pool(name="resident", bufs=1) as res,
            tc.tile_pool(name="psum", bufs=3, space="PSUM") as pp,
            tc.tile_pool(name="fold", bufs=4) as fp,
        ):
            # ---- PE warmup while DMA fills --------------------------------
            wsrc = res.tile([P, 512], bf16, tag="wsrc")
            nc.vector.memset(wsrc[:], 0.0)
            wp = pp.tile([P, MC], f32, tag="pblk", name="warm")
            for _ in range(8):
                nc.tensor.matmul(wp[:, 0:512], wsrc[:, 0:P], wsrc[:, :],
                                 start=True, stop=True)

            # ---- DMA fill (sync + gpsimd queues; scalar stays free) -------
            dma_engs = [nc.sync, nc.gpsimd]
            _rr = [0]

            def dma(out_ap, in_ap):
                dma_engs[_rr[0] % 2].dma_start(out=out_ap, in_=in_ap)
                _rr[0] += 1

            ut = res.tile([P, 2, MC], bf16, tag="umat")
            for h in (0, 1):
                dma(ut[:, h, :], umat[:, h * MC:(h + 1) * MC])
            ssqt_t = res.tile([P, T], f32, tag="ssqt")
            nc.scalar.dma_start(out=ssqt_t[:], in_=ssqt[:, :])

            vts = []       # per cand-tile: (chunk_tile, col_offset_in_chunk)
            coff = 0
            for ci, cw in enumerate(chunks):
                vt = res.tile([P, 2, cw * P], bf16, tag=f"v{ci}",
                              name=f"v{ci}")
                for h in (0, 1):
                    o = h * NCOLS + coff * P
                    dma(vt[:, h, :], vmat[:, o:o + cw * P])
                for j in range(cw):
                    vts.append((vt, j))
                coff += cw

            accP = res.tile([P, MC], f16, tag="accP")
            nc.vector.memset(accP[:], F16INIT)
            accN = res.tile([P, MC], f16, tag="accN")
            nc.vector.memset(accN[:], F16INIT)

            segs = [(0, 512), (512, 512)]

            # ---- main loop over candidate tiles ---------------------------
            pend = None  # (tmp, acc) waiting for its fp16 fold
            for t in range(T):
                vt, j = vts[t]
                psum = pp.tile([P, MC], f32, tag="pblk", name="pblk")
                for h in (0, 1):
                    stat = vt[:, h, j * P:(j + 1) * P]
                    for (s0, sw) in segs:
                        nc.tensor.matmul(
                            psum[:, s0:s0 + sw], stat, ut[:, h, s0:s0 + sw],
                            start=(h == 0), stop=(h == 1),
                        )
                acc = accP if t < tpos else accN
                tmp = fp.tile([P, FA], f16, tag="fold")
                nc.scalar.activation(tmp[:], psum[:, 0:FA], IDENT,
                                     bias=ssqt_t[:, t:t + 1], scale=1.0)
                nc.vector.scalar_tensor_tensor(
                    acc[:, FA:MC], in0=psum[:, FA:MC],
                    scalar=ssqt_t[:, t:t + 1],
                    in1=acc[:, FA:MC], op0=OP.add, op1=OP.min,
                )
                if pend is not None:
                    ptmp, pacc = pend
                    nc.vector.tensor_tensor(pacc[:, 0:FA], ptmp[:],
                                            pacc[:, 0:FA], op=OP.min)
                pend = (tmp, acc)
                if t == tpos + 1:
                    nc.sync.dma_start(out=out[:, 0:MC], in_=accP[:])

            ptmp, pacc = pend
            nc.vector.tensor_tensor(pacc[:, 0:FA], ptmp[:], pacc[:, 0:FA],
                                    op=OP.min)
            nc.sync.dma_start(out=out[:, MC:2 * MC], in_=accN[:])

    nc.compile()
    return nc


def _prepare_inputs(emb, lab, tpos, tneg):
    """Host-side shard/layout prep.

    Returns (in_maps, excess) where excess is the list of anchor indices
    (per class beyond the 1024-row cap) to be folded in on the host.
    """
    B = emb.shape[0]
    T = tpos + tneg
    NCOLS = T * P

    counts = np.bincount(lab, minlength=NCLS).astype(int)
    order = np.argsort(lab, kind="stable")
    cstart = np.concatenate([[0], np.cumsum(counts)]).astype(int)
    sq = np.einsum("ij,ij->i", emb, emb, dtype=np.float32)
    sqp = sq - np.float32(CENTER)

    in_maps = []
    excess = []
    meta = []
    for c in range(NCLS):
        own = order[cstart[c]:cstart[c + 1]]
        if len(own) == 0:
            own = order[0:1]
        aidx = own[:MC]
        excess.extend(own[MC:].tolist())
        aidx_p = np.concatenate(
            [aidx, np.full(MC - len(aidx), aidx[0], dtype=np.int64)])

        # candidate columns: own class (padded/capped) first, then others
        own_cap = own[:tpos * P]
        pos_cols = np.concatenate(
            [own_cap,
             np.full(tpos * P - len(own_cap), own[0], dtype=np.int64)])
        negs = np.concatenate(
            [order[cstart[c2]:cstart[c2 + 1]]
             for c2 in range(NCLS) if c2 != c and counts[c2] > 0])
        neg_cols = np.concatenate(
            [negs, np.full(tneg * P - len(negs), negs[0], dtype=np.int64)])
        colidx = np.concatenate([pos_cols, neg_cols])
        assert len(colidx) == NCOLS

        sign = np.ones(NCOLS, dtype=np.float32)
        sign[tpos * P:] = -1.0

        V = (emb[colidx] * sign[:, None]).astype(BF16).T  # [256, NCOLS]
        vm = np.empty((P, 2 * NCOLS), dtype=BF16)
        vm[:, :NCOLS] = V[0:P]
        vm[:, NCOLS:] = V[P:2 * P]

        U = (2.0 * emb[aidx_p]).astype(BF16).T            # [256, MC]
        um = np.empty((P, 2 * MC), dtype=BF16)
        um[:, :MC] = U[0:P]
        um[:, MC:] = U[P:2 * P]

        # per-tile scalar column: -sq' for pos tiles, +sq' for neg tiles
        ssqcol = np.where(sign > 0, -sqp[colidx], sqp[colidx])
        ssqt = ssqcol.reshape(T, P).T.copy()              # [128, T]

        in_maps.append({"vmat": vm, "umat": um, "ssqt": ssqt})
        meta.append((c, aidx, own[tpos * P:]))
    return in_maps, excess, meta


def _host_excess(emb, lab, excess):
    """Exact float64 triplet terms for anchors beyond the per-core cap."""
    if not excess:
        return 0.0, 0
    e64 = emb.astype(np.float64)
    sq64 = np.einsum("ij,ij->i", e64, e64)
    num = 0.0
    n = 0
    B = emb.shape[0]
    counts = np.bincount(lab, minlength=NCLS)
    for a in excess:
        c = lab[a]
        if not (2 <= counts[c] <= B - 1):
            continue
        d2 = sq64[a] + sq64 - 2.0 * (e64 @ e64[a])
        same = lab == c
        pos_d2 = np.max(np.where(same, d2, -np.inf))   # self d2=0 never max
        neg_d2 = np.min(np.where(same, np.inf, d2))
        per = np.sqrt(max(pos_d2, 0.0)) - np.sqrt(max(neg_d2, 0.0)) + MARGIN
        num += max(per, 0.0)
        n += 1
    return num, n


_PROGRAM_CACHE = {}


def _get_program(tpos, tneg):
    key = (tpos, tneg, FA)
    if key not in _PROGRAM_CACHE:
        _PROGRAM_CACHE[key] = _build_program(tpos, tneg)
    return _PROGRAM_CACHE[key]


def _combine(results, meta, emb, lab, counts, extra_num, extra_den):
    """Host epilogue: stats -> distances -> masked triplet loss."""
    B = emb.shape[0]
    sq = np.einsum("ij,ij->i", emb.astype(np.float64), emb.astype(np.float64))
    num = float(extra_num)
    den = float(extra_den)
    for r, (c, aidx, exc) in zip(results, meta):
        if not (2 <= counts[c] <= B - 1) or len(aidx) == 0:
            continue
        o = np.asarray(r["out"]).astype(np.float64)      # [128, 2*MC] fp16
        n = len(aidx)
        posst = o[:, 0:MC].min(axis=0)[:n]               # min over cand lanes
        negst = o[:, MC:2 * MC].min(axis=0)[:n]
        sqa = sq[aidx]
        pos_d2 = sqa + CENTER - posst
        neg_d2 = sqa + CENTER + negst
        if len(exc):                                     # uncovered positives
            d2x = (sqa[:, None] + sq[exc][None, :]
                   - 2.0 * emb[aidx].astype(np.float64)
                   @ emb[exc].astype(np.float64).T)
            pos_d2 = np.maximum(pos_d2, d2x.max(axis=1))
        per = (np.sqrt(np.maximum(pos_d2, 0.0))
               - np.sqrt(np.maximum(neg_d2, 0.0)) + MARGIN)
        num += np.maximum(per, 0.0).sum()
        den += n
    return np.float32(num / max(den, 1.0))


def _setup_trace_hook():
    import sys
    import types
    try:
        from antenv.axon_hooks import get_axon_ntff_profile_hook  # noqa: F401
        return
    except ImportError:
        pass
    import antenv
    from trn_agent_boot.trn_boot import _ntff_profile_via_ctypes

    mod = types.ModuleType("antenv.axon_hooks")
    state = {"h": None}
    mod.set_axon_ntff_profile_hook = lambda h: state.__setitem__("h", h)
    mod.get_axon_ntff_profile_hook = lambda: state["h"]
    sys.modules["antenv.axon_hooks"] = mod
    antenv.axon_hooks = mod
    mod.set_axon_ntff_profile_hook(
        _ntff_profile_via_ctypes("/opt/axon/libaxon_pjrt.so")
    )


def kernel(embeddings, labels, _trace=False):
    emb = np.ascontiguousarray(np.asarray(embeddings, dtype=np.float32))
    lab = np.asarray(labels).astype(np.int64).ravel()

    counts = np.bincount(lab, minlength=NCLS).astype(int)
    tpos, tneg = _layout(counts)
    nc = _get_program(tpos, tneg)
    in_maps, excess, meta = _prepare_inputs(emb, lab, tpos, tneg)
    extra_num, extra_den = _host_excess(emb, lab, excess)

    from concourse.bass_utils import run_bass_kernel_spmd

    if _trace:
        _setup_trace_hook()
        import concourse.bass_utils as _bu
        _bu.upload_artifacts = lambda tmpdir: tmpdir  # skip remote upload

    res = run_bass_kernel_spmd(
        nc, in_maps, core_ids=list(range(NCORES)), trace=bool(_trace),
    )
    loss = _combine(res.results, meta, emb, lab, counts,
                    extra_num, extra_den)
    if _trace:
        return loss, res
    return loss
